# revision 48
# baseline (speedup 1.0000x reference)
"""Multi-head attention (B=2, T=2048, C=1024, H=16, D=64) on 8 TRN2 cores.

Sharding: core c = 4*b + g handles batch b (2-way data parallel) and head
group g (4 heads, 4-way tensor parallel). qkv is column-parallel, proj is
row-parallel; the 4 partial proj outputs per batch are summed on host.

Device kernel (per core), all matmuls in bf16 with fp32 PSUM accumulate:
  qT = wq.T @ xT          [256, 2048]   (head dims on partitions)
  kT = wk.T @ xT          [256, 2048]
  v  = xT.T @ wv          [2048, 4, 65] (ones column appended per head)
  per head h, per 1024-wide query chunk:
    for each 128-wide key tile tk:
      scoresT = kT_h[:,tk].T @ qT_h     [128, 1024]  (keys on partitions)
      expT    = exp(scoresT / 8)        bf16
      pav    += vhat_h[tk].T @ expT     [65, 1024]   (row 64 = softmax denom)
    recip denom -> DRAM -> broadcast over 64 partitions -> attn_hT = num * r
  y = sum_h attn_hT.T @ wp_h            [2048, 1024] fp32 partial out
"""
import sys
import numpy as np

sys.path.insert(0, "/opt/trn_rl_repo")
import ml_dtypes

B, T, C = 2, 2048, 1024
NH, HD = 16, 64
HG = 4                    # heads per core
GC = HG * HD              # 256 columns per core
KT = C // 128             # 8 k-tiles for qkv contraction
TT = T // 128             # 16 token tiles
QC = 2                    # query chunks of 1024
QW = T // QC              # 1024
NCORES = 8

_cache = {}


def _dedup_ldweights(nc):
    """Drop InstLdweights identical to the immediately-previous PE weight
    load (only matmuls between), moving its waits onto the next matmul.
    The PE array keeps stationary weights across matmuls, so the reload is
    pure overhead (~107ns serialized; walrus ldw-opt is disabled)."""
    import concourse.mybir as mybir
    removed = 0
    for f in nc.m.functions:
        for bb in f.blocks:
            out = []
            prev_key = None
            pending = []
            for inst in bb.instructions:
                tn = type(inst).__name__
                if tn == "InstLdweights":
                    key = (str(inst.ins[0]), str(inst.is_transpose),
                           str(inst.perf_mode), str(inst.tile_position))
                    si = inst.sync_info
                    nw = len(si.on_wait) if si else 0
                    if (key == prev_key and not (si and si.on_update)
                            and nw == 0):
                        removed += 1
                        continue
                    prev_key = key
                elif tn in ("InstMatmult", "InstMatmultMx"):
                    if getattr(inst, "is_transpose", False):
                        prev_key = None
                    if pending:
                        si = inst.sync_info
                        inst.sync_info = mybir.SyncInfo(
                            on_wait=(list(si.on_wait) if si else []) + pending,
                            on_update=(list(si.on_update) if si else []))
                        pending = []
                elif tn in ("InstUnconditionalBranch", "InstCall",
                            "InstCompareBranch"):
                    prev_key = None
                out.append(inst)
            assert not pending
            bb.instructions[:] = out
    return removed


def _build(with_bias):
    import concourse.bass as bass
    import concourse.mybir as mybir
    import concourse.tile as tile
    from concourse import bacc

    f32 = mybir.dt.float32
    bf16 = mybir.dt.bfloat16

    nc = bacc.Bacc(None, target_bir_lowering=False)

    # weights arrive pre-arranged in SBUF layout (contiguous 2-4KB rows
    # per partition -> fast DMA descriptors)
    xt = nc.dram_tensor("xt", [C, T], bf16, kind="ExternalInput")
    wq = nc.dram_tensor("wq", [128, KT * GC], bf16, kind="ExternalInput")
    wk = nc.dram_tensor("wk", [128, KT * GC], bf16, kind="ExternalInput")
    wv = nc.dram_tensor("wv", [128, KT * GC], bf16, kind="ExternalInput")
    wp = nc.dram_tensor("wp", [128, 2 * C], bf16, kind="ExternalInput")
    if with_bias:
        bq = nc.dram_tensor("bq", [128, 2], f32, kind="ExternalInput")
        bk = nc.dram_tensor("bk", [128, 2], f32, kind="ExternalInput")
        bv = nc.dram_tensor("bv", [1, GC], f32, kind="ExternalInput")
    y = nc.dram_tensor("y", [T, C], f32, kind="ExternalOutput")

    with tile.TileContext(nc) as tc:
        with (
            tc.tile_pool(name="ins", bufs=1) as ins,
            tc.tile_pool(name="big", bufs=1) as bigp,
            tc.tile_pool(name="work", bufs=4) as work,
            tc.tile_pool(name="numsb", bufs=2) as numsb,
            tc.tile_pool(name="psa", bufs=1, space="PSUM") as psa,
            tc.tile_pool(name="psb", bufs=1, space="PSUM") as psb,
            tc.tile_pool(name="psav", bufs=1, space="PSUM") as psav,
            tc.tile_pool(name="psy", bufs=1, space="PSUM") as psy,
            tc.tile_pool(name="dram", bufs=8, space="DRAM") as dpool,
        ):
            # ---- input staging; spread over both HWDGE rings + SWDGE so
            # transfers overlap and the first q/k matmuls start ~8us in ----
            wq_sb = ins.tile([128, KT, GC], bf16, tag="wq")
            wk_sb = ins.tile([128, KT, GC], bf16, tag="wk")
            wv_sb = ins.tile([128, KT, GC], bf16, tag="wv")
            wp_sb = ins.tile([128, 2, C], bf16, tag="wp")
            xt_sb = ins.tile([128, KT, T], bf16, tag="xt")
            for kt in range(0, KT, 2):
                nc.sync.dma_start(xt_sb[:, kt, :], xt[kt * 128:(kt + 1) * 128, :])
            nc.scalar.dma_start(wq_sb[:], wq.rearrange("p (a n) -> p a n", a=KT))
            nc.scalar.dma_start(
                xt_sb[:, 1, :], xt[128:256, :])
            nc.scalar.dma_start(wk_sb[:], wk.rearrange("p (a n) -> p a n", a=KT))
            for kt in range(3, KT, 2):
                nc.scalar.dma_start(xt_sb[:, kt, :], xt[kt * 128:(kt + 1) * 128, :])
            nc.gpsimd.dma_start(wv_sb[:], wv.rearrange("p (a n) -> p a n", a=KT))
            nc.gpsimd.dma_start(wp_sb[:], wp.rearrange("p (a n) -> p a n", a=2))
            if with_bias:
                bq_sb = ins.tile([128, 2], f32, tag="bq")
                bk_sb = ins.tile([128, 2], f32, tag="bk")
                bv_sb = ins.tile([128, GC], f32, tag="bv")
                nc.gpsimd.dma_start(bq_sb[:], bq[:])
                nc.gpsimd.dma_start(bk_sb[:], bk[:])
                nc.gpsimd.dma_start(bv_sb[:], bv[0:1, :].to_broadcast([128, GC]))

            # ---- q/k/v projections. Accumulation groups rotate over 4 psum
            # slots (sc x2, av, py); 2 MMs per kt share one weight load ----
            qt_sb = bigp.tile([128, 2, T], bf16, tag="qt")
            kt_sb = bigp.tile([128, 2, T], bf16, tag="kt")
            _slot = [0]

            def qkv_psum():
                i = _slot[0] % 4
                _slot[0] += 1
                pool = (psa, psb, psav, psy)[i]
                tag = ("sca", "scb", "av", "py")[i]
                return pool.tile([128, 1024], f32, tag=tag, name="pqkv")

            def qk_group(mt):
                for src_sb, dst_sb, which in (
                    (wq_sb, qt_sb, "q"), (wk_sb, kt_sb, "k")):
                    for tsp in range(2):
                        pq = qkv_psum()
                        # descending kt from a late-arriving chunk: PE holds
                        # off until DMA nearly done, then runs gap-free (HAM
                        # stays warm once started)
                        kts = [(KT - 1 - _slot[0] - i) % KT for i in range(KT)]
                        for i, kt in enumerate(kts):
                            for half in range(2):
                                nc.tensor.matmul(
                                    pq[:, half * 512:(half + 1) * 512],
                                    src_sb[:, kt, mt * 128:(mt + 1) * 128],
                                    xt_sb[:, kt,
                                          tsp * 1024 + half * 512:
                                          tsp * 1024 + (half + 1) * 512],
                                    start=(i == 0), stop=(i == KT - 1),
                                )
                        dst = dst_sb[:, mt, tsp * 1024:(tsp + 1) * 1024]
                        if with_bias:
                            bias_sb = bq_sb if which == "q" else bk_sb
                            nc.vector.tensor_scalar_add(
                                dst, pq[:], bias_sb[:, mt:mt + 1])
                        else:
                            nc.vector.tensor_copy(dst, pq[:])

            qk_group(0)

            # ---- v projection (+ ones column for the softmax denominator) ----
            vhat_sb = bigp.tile([128, TT, HG, HD + 1], bf16, tag="vhat")
            nc.vector.memset(vhat_sb[:, :, :, HD:HD + 1], 1.0)
            for tt in range(TT):
                pv = qkv_psum()
                kts = [(tt + i) % KT for i in range(KT)]
                for i, kt in enumerate(kts):
                    nc.tensor.matmul(
                        pv[:, 0:GC],
                        xt_sb[:, kt, tt * 128:(tt + 1) * 128],
                        wv_sb[:, kt, :],
                        start=(i == 0), stop=(i == KT - 1),
                    )
                if with_bias:
                    nc.vector.tensor_add(
                        vhat_sb[:, tt, :, 0:HD], pv[:, 0:GC], bv_sb[:])
                else:
                    nc.vector.tensor_copy(vhat_sb[:, tt, :, 0:HD], pv[:, 0:GC])

            # ---- attention; proj groups interleave into the qc1 stream so
            # PE fills the ACT-bound bubble and y writeback overlaps ----
            # attn output packed as head pairs on full 128 partitions so the
            # proj matmuls contract K=128: even head -> partitions 0-63
            # (mul writes directly), odd head -> 64-127 via SBUF-SBUF DMA
            attn2_sb = bigp.tile([128, 2, T], bf16, tag="attn2")

            def attn_head(h):
                mt, off = h // 2, (h % 2) * 64
                pava = psav.tile([128, 1024], f32, tag="av", name="pava")
                pavb = psy.tile([128, 1024], f32, tag="py", name="pavb")
                for tk in range(TT):
                    psca = psa.tile([128, 1024], f32, tag="sca", name="psca")
                    pscb = psb.tile([128, 1024], f32, tag="scb", name="pscb")
                    # 4 scores MMs share one kT weight load (ldw dedup)
                    for qc, psc in ((0, psca), (1, pscb)):
                        for half in range(2):
                            nc.tensor.matmul(
                                psc[:, half * 512:(half + 1) * 512],
                                kt_sb[off:off + 64, mt, tk * 128:(tk + 1) * 128],
                                qt_sb[off:off + 64, mt,
                                      qc * QW + half * 512:
                                      qc * QW + (half + 1) * 512],
                                start=True, stop=True,
                            )
                    eta = work.tile([128, QW], bf16, tag="expt")
                    nc.scalar.activation(
                        eta[:], psca[:], mybir.ActivationFunctionType.Exp,
                        bias=0.0, scale=0.125)
                    etb = work.tile([128, QW], bf16, tag="expt")
                    nc.scalar.activation(
                        etb[:], pscb[:], mybir.ActivationFunctionType.Exp,
                        bias=0.0, scale=0.125)
                    # 4 AV MMs share one vhat weight load
                    for et, pav in ((eta, pava), (etb, pavb)):
                        for half in range(2):
                            nc.tensor.matmul(
                                pav[0:65, half * 512:(half + 1) * 512],
                                vhat_sb[:, tk, h, :],
                                et[:, half * 512:(half + 1) * 512],
                                start=(tk == 0), stop=(tk == TT - 1),
                            )
                for qc, pav in ((0, pava), (1, pavb)):
                    num = numsb.tile([65, QW], f32, tag="num", name="num")
                    nc.vector.tensor_copy(num[:], pav[0:65, :])
                    nc.vector.reciprocal(num[64:65, :], num[64:65, :])
                    dscr = dpool.tile([1, QW], f32, tag="den", name="dscr")
                    nc.gpsimd.dma_start(dscr[:], num[64:65, :])
                    rbc = numsb.tile([64, QW], f32, tag="rbc", name="rbc")
                    nc.gpsimd.dma_start(
                        rbc[:], dscr[0:1, :].to_broadcast([64, QW]))
                    if h % 2 == 0:
                        nc.vector.tensor_mul(
                            attn2_sb[0:64, h // 2, qc * QW:(qc + 1) * QW],
                            num[0:64, :], rbc[:])
                    else:
                        odd = numsb.tile([64, QW], bf16, tag="odd", name="odd")
                        nc.vector.tensor_mul(odd[:], num[0:64, :], rbc[:])
                        nc.gpsimd.dma_start(
                            attn2_sb[64:128, h // 2, qc * QW:(qc + 1) * QW],
                            odd[:])

            def proj_tt(tt):
                py = qkv_psum()
                for p in range(2):
                    for ns in range(2):
                        nc.tensor.matmul(
                            py[:, ns * 512:(ns + 1) * 512],
                            attn2_sb[:, p, tt * 128:(tt + 1) * 128],
                            wp_sb[:, p, ns * 512:(ns + 1) * 512],
                            start=(p == 0), stop=(p == 1),
                        )
                ysb = work.tile([128, 1024], f32, tag="ysb")
                nc.vector.tensor_copy(ysb[:], py[:])
                nc.sync.dma_start(y[tt * 128:(tt + 1) * 128, :], ysb[:])

            attn_head(0)
            attn_head(1)
            qk_group(1)
            attn_head(2)
            attn_head(3)
            for tt in range(TT):
                proj_tt(tt)

    nc.compile()
    _dedup_ldweights(nc)
    return nc


def _get_nc(with_bias):
    key = ("nc", with_bias)
    if key not in _cache:
        _cache[key] = _build(with_bias)
    return _cache[key]


def _sbuf_weight_layout(w, p):
    """[a*p, n] -> [p, a*n] matching sbuf tile [p, a, n]."""
    a = w.shape[0] // p
    return np.ascontiguousarray(
        w.reshape(a, p, w.shape[1]).transpose(1, 0, 2).reshape(p, -1))


def make_in_maps(x, w_qkv, b_qkv, w_proj, with_bias):
    bf = ml_dtypes.bfloat16
    x = np.asarray(x, dtype=np.float32)
    w_qkv = np.asarray(w_qkv, dtype=np.float32)
    b_qkv = np.asarray(b_qkv, dtype=np.float32)
    w_proj = np.asarray(w_proj, dtype=np.float32)
    in_maps = []
    for c in range(NCORES):
        b, g = divmod(c, HG)
        cols = slice(g * GC, (g + 1) * GC)
        m = {
            "xt": np.ascontiguousarray(x[b].T).astype(bf),
            "wq": _sbuf_weight_layout(
                w_qkv[:, 0 * C:1 * C][:, cols].astype(bf), 128),
            "wk": _sbuf_weight_layout(
                w_qkv[:, 1 * C:2 * C][:, cols].astype(bf), 128),
            "wv": _sbuf_weight_layout(
                w_qkv[:, 2 * C:3 * C][:, cols].astype(bf), 128),
            "wp": _sbuf_weight_layout(
                w_proj[g * GC:(g + 1) * GC, :].astype(bf), 128),
        }
        if with_bias:
            m["bq"] = np.ascontiguousarray(
                b_qkv[0 * C:1 * C][cols].reshape(2, 128).T).astype(np.float32)
            m["bk"] = np.ascontiguousarray(
                b_qkv[1 * C:2 * C][cols].reshape(2, 128).T).astype(np.float32)
            m["bv"] = np.ascontiguousarray(
                b_qkv[2 * C:3 * C][cols].reshape(1, GC)).astype(np.float32)
        in_maps.append(m)
    return in_maps


def gather(results, b_proj):
    b_proj = np.asarray(b_proj, dtype=np.float32)
    out = np.zeros((B, T, C), dtype=np.float32)
    for c in range(NCORES):
        b = c // HG
        out[b] += results[c]["y"]
    out += b_proj[None, None, :]
    return out


def kernel(x, w_qkv, b_qkv, w_proj, b_proj, _trace=False, _tmpdir=None):
    from concourse import bass_utils
    with_bias = bool(np.any(np.asarray(b_qkv)))
    nc = _get_nc(with_bias)
    in_maps = make_in_maps(x, w_qkv, b_qkv, w_proj, with_bias)
    res = bass_utils.run_bass_kernel_spmd(
        nc, in_maps, core_ids=list(range(NCORES)), trace=_trace,
        tmpdir=_tmpdir)
    _cache["last_result"] = res
    return gather(res.results, b_proj)


# revision 50
# speedup vs baseline: 1.1954x; 1.1954x over previous
"""Multi-head attention (B=2, T=2048, C=1024, H=16, D=64) on 8 TRN2 cores.

Sharding: core c = 4*b + g handles batch b (2-way data parallel) and head
group g (4 heads, 4-way tensor parallel). qkv is column-parallel, proj is
row-parallel; the 4 partial proj outputs per batch are summed on host.

Device kernel (per core), all matmuls in bf16 with fp32 PSUM accumulate:
  qT = wq.T @ xT          [256, 2048]   (head dims on partitions)
  kT = wk.T @ xT          [256, 2048]
  v  = xT.T @ wv          [2048, 4, 65] (ones column appended per head)
  per head h, per 1024-wide query chunk:
    for each 128-wide key tile tk:
      scoresT = kT_h[:,tk].T @ qT_h     [128, 1024]  (keys on partitions)
      expT    = exp(scoresT / 8)        bf16
      pav    += vhat_h[tk].T @ expT     [65, 1024]   (row 64 = softmax denom)
    recip denom -> DRAM -> broadcast over 64 partitions -> attn_hT = num * r
  y = sum_h attn_hT.T @ wp_h            [2048, 1024] fp32 partial out
"""
import sys
import numpy as np

sys.path.insert(0, "/opt/trn_rl_repo")
import ml_dtypes

B, T, C = 2, 2048, 1024
NH, HD = 16, 64
HG = 4                    # heads per core
GC = HG * HD              # 256 columns per core
KT = C // 128             # 8 k-tiles for qkv contraction
TT = T // 128             # 16 token tiles
QC = 2                    # query chunks of 1024
QW = T // QC              # 1024
NCORES = 8

_cache = {}


def _dedup_ldweights(nc):
    """Drop InstLdweights identical to the immediately-previous PE weight
    load (only matmuls between), moving its waits onto the next matmul.
    The PE array keeps stationary weights across matmuls, so the reload is
    pure overhead (~107ns serialized; walrus ldw-opt is disabled)."""
    import concourse.mybir as mybir
    removed = 0
    for f in nc.m.functions:
        for bb in f.blocks:
            out = []
            prev_key = None
            pending = []
            for inst in bb.instructions:
                tn = type(inst).__name__
                if tn == "InstLdweights":
                    key = (str(inst.ins[0]), str(inst.is_transpose),
                           str(inst.perf_mode), str(inst.tile_position))
                    si = inst.sync_info
                    nw = len(si.on_wait) if si else 0
                    if (key == prev_key and not (si and si.on_update)
                            and nw == 0):
                        removed += 1
                        continue
                    prev_key = key
                elif tn in ("InstMatmult", "InstMatmultMx"):
                    if getattr(inst, "is_transpose", False):
                        prev_key = None
                    if pending:
                        si = inst.sync_info
                        inst.sync_info = mybir.SyncInfo(
                            on_wait=(list(si.on_wait) if si else []) + pending,
                            on_update=(list(si.on_update) if si else []))
                        pending = []
                elif tn in ("InstUnconditionalBranch", "InstCall",
                            "InstCompareBranch"):
                    prev_key = None
                out.append(inst)
            assert not pending
            bb.instructions[:] = out
    return removed


def _build(with_bias):
    import concourse.bass as bass
    import concourse.mybir as mybir
    import concourse.tile as tile
    from concourse import bacc

    f32 = mybir.dt.float32
    bf16 = mybir.dt.bfloat16

    nc = bacc.Bacc(None, target_bir_lowering=False)

    # weights arrive pre-arranged in SBUF layout (contiguous 2-4KB rows
    # per partition -> fast DMA descriptors)
    xt = nc.dram_tensor("xt", [C, T], bf16, kind="ExternalInput")
    wq = nc.dram_tensor("wq", [128, KT * GC], bf16, kind="ExternalInput")
    wk = nc.dram_tensor("wk", [128, KT * GC], bf16, kind="ExternalInput")
    wv = nc.dram_tensor("wv", [128, KT * GC], bf16, kind="ExternalInput")
    wp = nc.dram_tensor("wp", [128, 2 * C], bf16, kind="ExternalInput")
    if with_bias:
        bq = nc.dram_tensor("bq", [128, 2], f32, kind="ExternalInput")
        bk = nc.dram_tensor("bk", [128, 2], f32, kind="ExternalInput")
        bv = nc.dram_tensor("bv", [1, GC], f32, kind="ExternalInput")
    y = nc.dram_tensor("y", [T, C], f32, kind="ExternalOutput")

    with tile.TileContext(nc) as tc:
        with (
            tc.tile_pool(name="ins", bufs=1) as ins,
            tc.tile_pool(name="big", bufs=1) as bigp,
            tc.tile_pool(name="work", bufs=4) as work,
            tc.tile_pool(name="numsb", bufs=2) as numsb,
            tc.tile_pool(name="psa", bufs=1, space="PSUM") as psa,
            tc.tile_pool(name="psb", bufs=1, space="PSUM") as psb,
            tc.tile_pool(name="psav", bufs=1, space="PSUM") as psav,
            tc.tile_pool(name="psy", bufs=1, space="PSUM") as psy,
            tc.tile_pool(name="dram", bufs=8, space="DRAM") as dpool,
        ):
            # ---- input staging; spread over both HWDGE rings + SWDGE so
            # transfers overlap and the first q/k matmuls start ~8us in ----
            wq_sb = ins.tile([128, KT, GC], bf16, tag="wq")
            wk_sb = ins.tile([128, KT, GC], bf16, tag="wk")
            wv_sb = ins.tile([128, KT, GC], bf16, tag="wv")
            wp_sb = ins.tile([128, 2, C], bf16, tag="wp")
            xt_sb = ins.tile([128, KT, T], bf16, tag="xt")
            for kt in range(0, KT, 2):
                nc.sync.dma_start(xt_sb[:, kt, :], xt[kt * 128:(kt + 1) * 128, :])
            nc.scalar.dma_start(wq_sb[:], wq.rearrange("p (a n) -> p a n", a=KT))
            nc.scalar.dma_start(
                xt_sb[:, 1, :], xt[128:256, :])
            nc.scalar.dma_start(wk_sb[:], wk.rearrange("p (a n) -> p a n", a=KT))
            for kt in range(3, KT, 2):
                nc.scalar.dma_start(xt_sb[:, kt, :], xt[kt * 128:(kt + 1) * 128, :])
            nc.gpsimd.dma_start(wv_sb[:], wv.rearrange("p (a n) -> p a n", a=KT))
            nc.gpsimd.dma_start(wp_sb[:], wp.rearrange("p (a n) -> p a n", a=2))
            if with_bias:
                bq_sb = ins.tile([128, 2], f32, tag="bq")
                bk_sb = ins.tile([128, 2], f32, tag="bk")
                bv_sb = ins.tile([128, GC], f32, tag="bv")
                nc.gpsimd.dma_start(bq_sb[:], bq[:])
                nc.gpsimd.dma_start(bk_sb[:], bk[:])
                nc.gpsimd.dma_start(bv_sb[:], bv[0:1, :].to_broadcast([128, GC]))

            # ---- q/k/v projections. Accumulation groups rotate over 4 psum
            # slots (sc x2, av, py); 2 MMs per kt share one weight load ----
            qt_sb = bigp.tile([128, 2, T], bf16, tag="qt")
            kt_sb = bigp.tile([128, 2, T], bf16, tag="kt")
            _slot = [0]

            def qkv_psum():
                i = _slot[0] % 4
                _slot[0] += 1
                pool = (psa, psb, psav, psy)[i]
                tag = ("sca", "scb", "av", "py")[i]
                return pool.tile([128, 1024], f32, tag=tag, name="pqkv")

            def qk_group(mt):
                for src_sb, dst_sb, which in (
                    (wq_sb, qt_sb, "q"), (wk_sb, kt_sb, "k")):
                    for tsp in range(2):
                        pq = qkv_psum()
                        # descending kt from a late-arriving chunk: PE holds
                        # off until DMA nearly done, then runs gap-free (HAM
                        # stays warm once started)
                        kts = [(KT - 1 - _slot[0] - i) % KT for i in range(KT)]
                        for i, kt in enumerate(kts):
                            for half in range(2):
                                nc.tensor.matmul(
                                    pq[:, half * 512:(half + 1) * 512],
                                    src_sb[:, kt, mt * 128:(mt + 1) * 128],
                                    xt_sb[:, kt,
                                          tsp * 1024 + half * 512:
                                          tsp * 1024 + (half + 1) * 512],
                                    start=(i == 0), stop=(i == KT - 1),
                                )
                        dst = dst_sb[:, mt, tsp * 1024:(tsp + 1) * 1024]
                        if with_bias:
                            bias_sb = bq_sb if which == "q" else bk_sb
                            nc.vector.tensor_scalar_add(
                                dst, pq[:], bias_sb[:, mt:mt + 1])
                        else:
                            nc.vector.tensor_copy(dst, pq[:])

            qk_group(0)

            # ---- v projection (+ ones column for the softmax denominator) ----
            vhat_sb = bigp.tile([128, TT, HG, HD + 1], bf16, tag="vhat")
            nc.vector.memset(vhat_sb[:, :, :, HD:HD + 1], 1.0)
            for tt in range(TT):
                pv = qkv_psum()
                kts = [(tt + i) % KT for i in range(KT)]
                for i, kt in enumerate(kts):
                    nc.tensor.matmul(
                        pv[:, 0:GC],
                        xt_sb[:, kt, tt * 128:(tt + 1) * 128],
                        wv_sb[:, kt, :],
                        start=(i == 0), stop=(i == KT - 1),
                    )
                if with_bias:
                    nc.vector.tensor_add(
                        vhat_sb[:, tt, :, 0:HD], pv[:, 0:GC], bv_sb[:])
                else:
                    nc.vector.tensor_copy(vhat_sb[:, tt, :, 0:HD], pv[:, 0:GC])

            qk_group(1)

            # ---- attention; proj groups interleave into the qc1 stream so
            # PE fills the ACT-bound bubble and y writeback overlaps ----
            # attn output packed as head pairs on full 128 partitions so the
            # proj matmuls contract K=128: even head -> partitions 0-63
            # (mul writes directly), odd head -> 64-127 via SBUF-SBUF DMA
            attn2_sb = bigp.tile([128, 2, T], bf16, tag="attn2")

            def attn_head(h):
                mt, off = h // 2, (h % 2) * 64
                pava = psav.tile([128, 1024], f32, tag="av", name="pava")
                pavb = psy.tile([128, 1024], f32, tag="py", name="pavb")
                for tk in range(TT):
                    psca = psa.tile([128, 1024], f32, tag="sca", name="psca")
                    pscb = psb.tile([128, 1024], f32, tag="scb", name="pscb")
                    # 4 scores MMs share one kT weight load (ldw dedup)
                    for qc, psc in ((0, psca), (1, pscb)):
                        for half in range(2):
                            nc.tensor.matmul(
                                psc[:, half * 512:(half + 1) * 512],
                                kt_sb[off:off + 64, mt, tk * 128:(tk + 1) * 128],
                                qt_sb[off:off + 64, mt,
                                      qc * QW + half * 512:
                                      qc * QW + (half + 1) * 512],
                                start=True, stop=True,
                            )
                    eta = work.tile([128, QW], bf16, tag="expt")
                    nc.scalar.activation(
                        eta[:], psca[:], mybir.ActivationFunctionType.Exp,
                        bias=0.0, scale=0.125)
                    etb = work.tile([128, QW], bf16, tag="expt")
                    nc.scalar.activation(
                        etb[:], pscb[:], mybir.ActivationFunctionType.Exp,
                        bias=0.0, scale=0.125)
                    # 4 AV MMs share one vhat weight load
                    for et, pav in ((eta, pava), (etb, pavb)):
                        for half in range(2):
                            nc.tensor.matmul(
                                pav[0:65, half * 512:(half + 1) * 512],
                                vhat_sb[:, tk, h, :],
                                et[:, half * 512:(half + 1) * 512],
                                start=(tk == 0), stop=(tk == TT - 1),
                            )
                for qc, pav in ((0, pava), (1, pavb)):
                    num = numsb.tile([65, QW], f32, tag="num", name="num")
                    nc.vector.tensor_copy(num[:], pav[0:65, :])
                    nc.vector.reciprocal(num[64:65, :], num[64:65, :])
                    dscr = dpool.tile([1, QW], f32, tag="den", name="dscr")
                    nc.gpsimd.dma_start(dscr[:], num[64:65, :])
                    rbc = numsb.tile([64, QW], f32, tag="rbc", name="rbc")
                    nc.gpsimd.dma_start(
                        rbc[:], dscr[0:1, :].to_broadcast([64, QW]))
                    if h % 2 == 0:
                        nc.vector.tensor_mul(
                            attn2_sb[0:64, h // 2, qc * QW:(qc + 1) * QW],
                            num[0:64, :], rbc[:])
                    else:
                        odd = numsb.tile([64, QW], bf16, tag="odd", name="odd")
                        nc.vector.tensor_mul(odd[:], num[0:64, :], rbc[:])
                        nc.gpsimd.dma_start(
                            attn2_sb[64:128, h // 2, qc * QW:(qc + 1) * QW],
                            odd[:])

            def proj_tt(tt):
                py = qkv_psum()
                for p in range(2):
                    for ns in range(2):
                        nc.tensor.matmul(
                            py[:, ns * 512:(ns + 1) * 512],
                            attn2_sb[:, p, tt * 128:(tt + 1) * 128],
                            wp_sb[:, p, ns * 512:(ns + 1) * 512],
                            start=(p == 0), stop=(p == 1),
                        )
                ysb = work.tile([128, 1024], f32, tag="ysb")
                nc.vector.tensor_copy(ysb[:], py[:])
                nc.sync.dma_start(y[tt * 128:(tt + 1) * 128, :], ysb[:])

            for h in range(HG):
                attn_head(h)
            for tt in range(TT):
                proj_tt(tt)

    nc.compile()
    _dedup_ldweights(nc)
    return nc


def _get_nc(with_bias):
    key = ("nc", with_bias)
    if key not in _cache:
        _cache[key] = _build(with_bias)
    return _cache[key]


def _sbuf_weight_layout(w, p):
    """[a*p, n] -> [p, a*n] matching sbuf tile [p, a, n]."""
    a = w.shape[0] // p
    return np.ascontiguousarray(
        w.reshape(a, p, w.shape[1]).transpose(1, 0, 2).reshape(p, -1))


def make_in_maps(x, w_qkv, b_qkv, w_proj, with_bias):
    bf = ml_dtypes.bfloat16
    x = np.asarray(x, dtype=np.float32)
    w_qkv = np.asarray(w_qkv, dtype=np.float32)
    b_qkv = np.asarray(b_qkv, dtype=np.float32)
    w_proj = np.asarray(w_proj, dtype=np.float32)
    in_maps = []
    for c in range(NCORES):
        b, g = divmod(c, HG)
        cols = slice(g * GC, (g + 1) * GC)
        m = {
            "xt": np.ascontiguousarray(x[b].T).astype(bf),
            "wq": _sbuf_weight_layout(
                w_qkv[:, 0 * C:1 * C][:, cols].astype(bf), 128),
            "wk": _sbuf_weight_layout(
                w_qkv[:, 1 * C:2 * C][:, cols].astype(bf), 128),
            "wv": _sbuf_weight_layout(
                w_qkv[:, 2 * C:3 * C][:, cols].astype(bf), 128),
            "wp": _sbuf_weight_layout(
                w_proj[g * GC:(g + 1) * GC, :].astype(bf), 128),
        }
        if with_bias:
            m["bq"] = np.ascontiguousarray(
                b_qkv[0 * C:1 * C][cols].reshape(2, 128).T).astype(np.float32)
            m["bk"] = np.ascontiguousarray(
                b_qkv[1 * C:2 * C][cols].reshape(2, 128).T).astype(np.float32)
            m["bv"] = np.ascontiguousarray(
                b_qkv[2 * C:3 * C][cols].reshape(1, GC)).astype(np.float32)
        in_maps.append(m)
    return in_maps


def gather(results, b_proj):
    b_proj = np.asarray(b_proj, dtype=np.float32)
    out = np.zeros((B, T, C), dtype=np.float32)
    for c in range(NCORES):
        b = c // HG
        out[b] += results[c]["y"]
    out += b_proj[None, None, :]
    return out


def kernel(x, w_qkv, b_qkv, w_proj, b_proj, _trace=False, _tmpdir=None):
    from concourse import bass_utils
    with_bias = bool(np.any(np.asarray(b_qkv)))
    nc = _get_nc(with_bias)
    in_maps = make_in_maps(x, w_qkv, b_qkv, w_proj, with_bias)
    res = bass_utils.run_bass_kernel_spmd(
        nc, in_maps, core_ids=list(range(NCORES)), trace=_trace,
        tmpdir=_tmpdir)
    _cache["last_result"] = res
    return gather(res.results, b_proj)


# revision 51
# speedup vs baseline: 1.2197x; 1.0203x over previous
"""Multi-head attention (B=2, T=2048, C=1024, H=16, D=64) on 8 TRN2 cores.

Sharding: core c = 4*b + g handles batch b (2-way data parallel) and head
group g (4 heads, 4-way tensor parallel). qkv is column-parallel, proj is
row-parallel; the 4 partial proj outputs per batch are summed on host.

Device kernel (per core), all matmuls in bf16 with fp32 PSUM accumulate:
  qT = wq.T @ xT          [256, 2048]   (head dims on partitions)
  kT = wk.T @ xT          [256, 2048]
  v  = xT.T @ wv          [2048, 4, 65] (ones column appended per head)
  per head h, per 1024-wide query chunk:
    for each 128-wide key tile tk:
      scoresT = kT_h[:,tk].T @ qT_h     [128, 1024]  (keys on partitions)
      expT    = exp(scoresT / 8)        bf16
      pav    += vhat_h[tk].T @ expT     [65, 1024]   (row 64 = softmax denom)
    recip denom -> DRAM -> broadcast over 64 partitions -> attn_hT = num * r
  y = sum_h attn_hT.T @ wp_h            [2048, 1024] fp32 partial out
"""
import sys
import numpy as np

sys.path.insert(0, "/opt/trn_rl_repo")
import ml_dtypes

B, T, C = 2, 2048, 1024
NH, HD = 16, 64
HG = 4                    # heads per core
GC = HG * HD              # 256 columns per core
KT = C // 128             # 8 k-tiles for qkv contraction
TT = T // 128             # 16 token tiles
QC = 2                    # query chunks of 1024
QW = T // QC              # 1024
NCORES = 8

_cache = {}


def _dedup_ldweights(nc):
    """Drop InstLdweights identical to the immediately-previous PE weight
    load (only matmuls between), moving its waits onto the next matmul.
    The PE array keeps stationary weights across matmuls, so the reload is
    pure overhead (~107ns serialized; walrus ldw-opt is disabled)."""
    import concourse.mybir as mybir
    removed = 0
    for f in nc.m.functions:
        for bb in f.blocks:
            out = []
            prev_key = None
            pending = []
            for inst in bb.instructions:
                tn = type(inst).__name__
                if tn == "InstLdweights":
                    key = (str(inst.ins[0]), str(inst.is_transpose),
                           str(inst.perf_mode), str(inst.tile_position))
                    si = inst.sync_info
                    nw = len(si.on_wait) if si else 0
                    if (key == prev_key and not (si and si.on_update)
                            and nw == 0):
                        removed += 1
                        continue
                    prev_key = key
                elif tn in ("InstMatmult", "InstMatmultMx"):
                    if getattr(inst, "is_transpose", False):
                        prev_key = None
                    if pending:
                        si = inst.sync_info
                        inst.sync_info = mybir.SyncInfo(
                            on_wait=(list(si.on_wait) if si else []) + pending,
                            on_update=(list(si.on_update) if si else []))
                        pending = []
                elif tn in ("InstUnconditionalBranch", "InstCall",
                            "InstCompareBranch"):
                    prev_key = None
                out.append(inst)
            assert not pending
            bb.instructions[:] = out
    return removed


def _build(with_bias):
    import concourse.bass as bass
    import concourse.mybir as mybir
    import concourse.tile as tile
    from concourse import bacc

    f32 = mybir.dt.float32
    bf16 = mybir.dt.bfloat16

    nc = bacc.Bacc(None, target_bir_lowering=False)

    # weights arrive pre-arranged in SBUF layout (contiguous 2-4KB rows
    # per partition -> fast DMA descriptors)
    xt = nc.dram_tensor("xt", [C, T], bf16, kind="ExternalInput")
    wq = nc.dram_tensor("wq", [128, KT * GC], bf16, kind="ExternalInput")
    wk = nc.dram_tensor("wk", [128, KT * GC], bf16, kind="ExternalInput")
    wv = nc.dram_tensor("wv", [128, KT * GC], bf16, kind="ExternalInput")
    wp = nc.dram_tensor("wp", [128, 2 * C], bf16, kind="ExternalInput")
    if with_bias:
        bq = nc.dram_tensor("bq", [128, 2], f32, kind="ExternalInput")
        bk = nc.dram_tensor("bk", [128, 2], f32, kind="ExternalInput")
        bv = nc.dram_tensor("bv", [1, GC], f32, kind="ExternalInput")
    y = nc.dram_tensor("y", [T, C], f32, kind="ExternalOutput")

    with tile.TileContext(nc) as tc:
        with (
            tc.tile_pool(name="ins", bufs=1) as ins,
            tc.tile_pool(name="big", bufs=1) as bigp,
            tc.tile_pool(name="work", bufs=4) as work,
            tc.tile_pool(name="numsb", bufs=2) as numsb,
            tc.tile_pool(name="psa", bufs=1, space="PSUM") as psa,
            tc.tile_pool(name="psb", bufs=1, space="PSUM") as psb,
            tc.tile_pool(name="psav", bufs=1, space="PSUM") as psav,
            tc.tile_pool(name="psy", bufs=1, space="PSUM") as psy,
            tc.tile_pool(name="dram", bufs=8, space="DRAM") as dpool,
        ):
            # ---- input staging; spread over both HWDGE rings + SWDGE so
            # transfers overlap and the first q/k matmuls start ~8us in ----
            wq_sb = ins.tile([128, KT, GC], bf16, tag="wq")
            wk_sb = ins.tile([128, KT, GC], bf16, tag="wk")
            wv_sb = ins.tile([128, KT, GC], bf16, tag="wv")
            wp_sb = ins.tile([128, 2, C], bf16, tag="wp")
            xt_sb = ins.tile([128, KT, T], bf16, tag="xt")
            # chunks arrive in DESCENDING kt order, matching the descending
            # kt-stagger of the accumulation groups below: PE runs gap-free
            # from the first chunk instead of waiting for the last
            for kt in (7, 5, 3, 1):
                nc.sync.dma_start(xt_sb[:, kt, :], xt[kt * 128:(kt + 1) * 128, :])
            nc.scalar.dma_start(wq_sb[:], wq.rearrange("p (a n) -> p a n", a=KT))
            nc.scalar.dma_start(wk_sb[:], wk.rearrange("p (a n) -> p a n", a=KT))
            for kt in (6, 4, 2, 0):
                nc.scalar.dma_start(xt_sb[:, kt, :], xt[kt * 128:(kt + 1) * 128, :])
            nc.gpsimd.dma_start(wv_sb[:], wv.rearrange("p (a n) -> p a n", a=KT))
            nc.gpsimd.dma_start(wp_sb[:], wp.rearrange("p (a n) -> p a n", a=2))
            if with_bias:
                bq_sb = ins.tile([128, 2], f32, tag="bq")
                bk_sb = ins.tile([128, 2], f32, tag="bk")
                bv_sb = ins.tile([128, GC], f32, tag="bv")
                nc.gpsimd.dma_start(bq_sb[:], bq[:])
                nc.gpsimd.dma_start(bk_sb[:], bk[:])
                nc.gpsimd.dma_start(bv_sb[:], bv[0:1, :].to_broadcast([128, GC]))

            # ---- q/k/v projections. Accumulation groups rotate over 4 psum
            # slots (sc x2, av, py); 2 MMs per kt share one weight load ----
            qt_sb = bigp.tile([128, 2, T], bf16, tag="qt")
            kt_sb = bigp.tile([128, 2, T], bf16, tag="kt")
            _slot = [0]

            def qkv_psum():
                i = _slot[0] % 4
                _slot[0] += 1
                pool = (psa, psb, psav, psy)[i]
                tag = ("sca", "scb", "av", "py")[i]
                return pool.tile([128, 1024], f32, tag=tag, name="pqkv")

            def qk_group(mt):
                for src_sb, dst_sb, which in (
                    (wq_sb, qt_sb, "q"), (wk_sb, kt_sb, "k")):
                    for tsp in range(2):
                        pq = qkv_psum()
                        # descending kt from a late-arriving chunk: PE holds
                        # off until DMA nearly done, then runs gap-free (HAM
                        # stays warm once started)
                        kts = [(KT - 1 - _slot[0] - i) % KT for i in range(KT)]
                        for i, kt in enumerate(kts):
                            for half in range(2):
                                nc.tensor.matmul(
                                    pq[:, half * 512:(half + 1) * 512],
                                    src_sb[:, kt, mt * 128:(mt + 1) * 128],
                                    xt_sb[:, kt,
                                          tsp * 1024 + half * 512:
                                          tsp * 1024 + (half + 1) * 512],
                                    start=(i == 0), stop=(i == KT - 1),
                                )
                        dst = dst_sb[:, mt, tsp * 1024:(tsp + 1) * 1024]
                        if with_bias:
                            bias_sb = bq_sb if which == "q" else bk_sb
                            nc.vector.tensor_scalar_add(
                                dst, pq[:], bias_sb[:, mt:mt + 1])
                        else:
                            nc.vector.tensor_copy(dst, pq[:])

            qk_group(0)

            # ---- v projection (+ ones column for the softmax denominator) ----
            vhat_sb = bigp.tile([128, TT, HG, HD + 1], bf16, tag="vhat")
            nc.vector.memset(vhat_sb[:, :, :, HD:HD + 1], 1.0)
            for tt in range(TT):
                pv = qkv_psum()
                kts = [(tt + i) % KT for i in range(KT)]
                for i, kt in enumerate(kts):
                    nc.tensor.matmul(
                        pv[:, 0:GC],
                        xt_sb[:, kt, tt * 128:(tt + 1) * 128],
                        wv_sb[:, kt, :],
                        start=(i == 0), stop=(i == KT - 1),
                    )
                if with_bias:
                    nc.vector.tensor_add(
                        vhat_sb[:, tt, :, 0:HD], pv[:, 0:GC], bv_sb[:])
                else:
                    nc.vector.tensor_copy(vhat_sb[:, tt, :, 0:HD], pv[:, 0:GC])

            qk_group(1)

            # ---- attention; proj groups interleave into the qc1 stream so
            # PE fills the ACT-bound bubble and y writeback overlaps ----
            # attn output packed as head pairs on full 128 partitions so the
            # proj matmuls contract K=128: even head -> partitions 0-63
            # (mul writes directly), odd head -> 64-127 via SBUF-SBUF DMA
            attn2_sb = bigp.tile([128, 2, T], bf16, tag="attn2")

            def attn_head(h):
                mt, off = h // 2, (h % 2) * 64
                pava = psav.tile([128, 1024], f32, tag="av", name="pava")
                pavb = psy.tile([128, 1024], f32, tag="py", name="pavb")
                for tk in range(TT):
                    psca = psa.tile([128, 1024], f32, tag="sca", name="psca")
                    pscb = psb.tile([128, 1024], f32, tag="scb", name="pscb")
                    # 4 scores MMs share one kT weight load (ldw dedup)
                    for qc, psc in ((0, psca), (1, pscb)):
                        for half in range(2):
                            nc.tensor.matmul(
                                psc[:, half * 512:(half + 1) * 512],
                                kt_sb[off:off + 64, mt, tk * 128:(tk + 1) * 128],
                                qt_sb[off:off + 64, mt,
                                      qc * QW + half * 512:
                                      qc * QW + (half + 1) * 512],
                                start=True, stop=True,
                            )
                    eta = work.tile([128, QW], bf16, tag="expt")
                    nc.scalar.activation(
                        eta[:], psca[:], mybir.ActivationFunctionType.Exp,
                        bias=0.0, scale=0.125)
                    etb = work.tile([128, QW], bf16, tag="expt")
                    nc.scalar.activation(
                        etb[:], pscb[:], mybir.ActivationFunctionType.Exp,
                        bias=0.0, scale=0.125)
                    # 4 AV MMs share one vhat weight load
                    for et, pav in ((eta, pava), (etb, pavb)):
                        for half in range(2):
                            nc.tensor.matmul(
                                pav[0:65, half * 512:(half + 1) * 512],
                                vhat_sb[:, tk, h, :],
                                et[:, half * 512:(half + 1) * 512],
                                start=(tk == 0), stop=(tk == TT - 1),
                            )
                for qc, pav in ((0, pava), (1, pavb)):
                    num = numsb.tile([65, QW], f32, tag="num", name="num")
                    nc.vector.tensor_copy(num[:], pav[0:65, :])
                    nc.vector.reciprocal(num[64:65, :], num[64:65, :])
                    dscr = dpool.tile([1, QW], f32, tag="den", name="dscr")
                    nc.gpsimd.dma_start(dscr[:], num[64:65, :])
                    rbc = numsb.tile([64, QW], f32, tag="rbc", name="rbc")
                    nc.gpsimd.dma_start(
                        rbc[:], dscr[0:1, :].to_broadcast([64, QW]))
                    if h % 2 == 0:
                        nc.vector.tensor_mul(
                            attn2_sb[0:64, h // 2, qc * QW:(qc + 1) * QW],
                            num[0:64, :], rbc[:])
                    else:
                        odd = numsb.tile([64, QW], bf16, tag="odd", name="odd")
                        nc.vector.tensor_mul(odd[:], num[0:64, :], rbc[:])
                        nc.gpsimd.dma_start(
                            attn2_sb[64:128, h // 2, qc * QW:(qc + 1) * QW],
                            odd[:])

            def proj_tt(tt):
                py = qkv_psum()
                for p in range(2):
                    for ns in range(2):
                        nc.tensor.matmul(
                            py[:, ns * 512:(ns + 1) * 512],
                            attn2_sb[:, p, tt * 128:(tt + 1) * 128],
                            wp_sb[:, p, ns * 512:(ns + 1) * 512],
                            start=(p == 0), stop=(p == 1),
                        )
                ysb = work.tile([128, 1024], f32, tag="ysb")
                nc.vector.tensor_copy(ysb[:], py[:])
                nc.sync.dma_start(y[tt * 128:(tt + 1) * 128, :], ysb[:])

            for h in range(HG):
                attn_head(h)
            for tt in range(TT):
                proj_tt(tt)

    nc.compile()
    _dedup_ldweights(nc)
    return nc


def _get_nc(with_bias):
    key = ("nc", with_bias)
    if key not in _cache:
        _cache[key] = _build(with_bias)
    return _cache[key]


def _sbuf_weight_layout(w, p):
    """[a*p, n] -> [p, a*n] matching sbuf tile [p, a, n]."""
    a = w.shape[0] // p
    return np.ascontiguousarray(
        w.reshape(a, p, w.shape[1]).transpose(1, 0, 2).reshape(p, -1))


def make_in_maps(x, w_qkv, b_qkv, w_proj, with_bias):
    bf = ml_dtypes.bfloat16
    x = np.asarray(x, dtype=np.float32)
    w_qkv = np.asarray(w_qkv, dtype=np.float32)
    b_qkv = np.asarray(b_qkv, dtype=np.float32)
    w_proj = np.asarray(w_proj, dtype=np.float32)
    in_maps = []
    for c in range(NCORES):
        b, g = divmod(c, HG)
        cols = slice(g * GC, (g + 1) * GC)
        m = {
            "xt": np.ascontiguousarray(x[b].T).astype(bf),
            "wq": _sbuf_weight_layout(
                w_qkv[:, 0 * C:1 * C][:, cols].astype(bf), 128),
            "wk": _sbuf_weight_layout(
                w_qkv[:, 1 * C:2 * C][:, cols].astype(bf), 128),
            "wv": _sbuf_weight_layout(
                w_qkv[:, 2 * C:3 * C][:, cols].astype(bf), 128),
            "wp": _sbuf_weight_layout(
                w_proj[g * GC:(g + 1) * GC, :].astype(bf), 128),
        }
        if with_bias:
            m["bq"] = np.ascontiguousarray(
                b_qkv[0 * C:1 * C][cols].reshape(2, 128).T).astype(np.float32)
            m["bk"] = np.ascontiguousarray(
                b_qkv[1 * C:2 * C][cols].reshape(2, 128).T).astype(np.float32)
            m["bv"] = np.ascontiguousarray(
                b_qkv[2 * C:3 * C][cols].reshape(1, GC)).astype(np.float32)
        in_maps.append(m)
    return in_maps


def gather(results, b_proj):
    b_proj = np.asarray(b_proj, dtype=np.float32)
    out = np.zeros((B, T, C), dtype=np.float32)
    for c in range(NCORES):
        b = c // HG
        out[b] += results[c]["y"]
    out += b_proj[None, None, :]
    return out


def kernel(x, w_qkv, b_qkv, w_proj, b_proj, _trace=False, _tmpdir=None):
    from concourse import bass_utils
    with_bias = bool(np.any(np.asarray(b_qkv)))
    nc = _get_nc(with_bias)
    in_maps = make_in_maps(x, w_qkv, b_qkv, w_proj, with_bias)
    res = bass_utils.run_bass_kernel_spmd(
        nc, in_maps, core_ids=list(range(NCORES)), trace=_trace,
        tmpdir=_tmpdir)
    _cache["last_result"] = res
    return gather(res.results, b_proj)


# revision 52
# speedup vs baseline: 1.2357x; 1.0131x over previous
"""Multi-head attention (B=2, T=2048, C=1024, H=16, D=64) on 8 TRN2 cores.

Sharding: core c = 4*b + g handles batch b (2-way data parallel) and head
group g (4 heads, 4-way tensor parallel). qkv is column-parallel, proj is
row-parallel; the 4 partial proj outputs per batch are summed on host.

Device kernel (per core), all matmuls in bf16 with fp32 PSUM accumulate:
  qT = wq.T @ xT          [256, 2048]   (head dims on partitions)
  kT = wk.T @ xT          [256, 2048]
  v  = xT.T @ wv          [2048, 4, 65] (ones column appended per head)
  per head h, per 1024-wide query chunk:
    for each 128-wide key tile tk:
      scoresT = kT_h[:,tk].T @ qT_h     [128, 1024]  (keys on partitions)
      expT    = exp(scoresT / 8)        bf16
      pav    += vhat_h[tk].T @ expT     [65, 1024]   (row 64 = softmax denom)
    recip denom -> DRAM -> broadcast over 64 partitions -> attn_hT = num * r
  y = sum_h attn_hT.T @ wp_h            [2048, 1024] fp32 partial out
"""
import sys
import numpy as np

sys.path.insert(0, "/opt/trn_rl_repo")
import ml_dtypes

B, T, C = 2, 2048, 1024
NH, HD = 16, 64
HG = 4                    # heads per core
GC = HG * HD              # 256 columns per core
KT = C // 128             # 8 k-tiles for qkv contraction
TT = T // 128             # 16 token tiles
QC = 2                    # query chunks of 1024
QW = T // QC              # 1024
NCORES = 8

_cache = {}


def _dedup_ldweights(nc):
    """Drop InstLdweights identical to the immediately-previous PE weight
    load (only matmuls between), moving its waits onto the next matmul.
    The PE array keeps stationary weights across matmuls, so the reload is
    pure overhead (~107ns serialized; walrus ldw-opt is disabled)."""
    import concourse.mybir as mybir
    removed = 0
    for f in nc.m.functions:
        for bb in f.blocks:
            out = []
            prev_key = None
            pending = []
            for inst in bb.instructions:
                tn = type(inst).__name__
                if tn == "InstLdweights":
                    key = (str(inst.ins[0]), str(inst.is_transpose),
                           str(inst.perf_mode), str(inst.tile_position))
                    si = inst.sync_info
                    nw = len(si.on_wait) if si else 0
                    if (key == prev_key and not (si and si.on_update)
                            and nw == 0):
                        removed += 1
                        continue
                    prev_key = key
                elif tn in ("InstMatmult", "InstMatmultMx"):
                    if getattr(inst, "is_transpose", False):
                        prev_key = None
                    if pending:
                        si = inst.sync_info
                        inst.sync_info = mybir.SyncInfo(
                            on_wait=(list(si.on_wait) if si else []) + pending,
                            on_update=(list(si.on_update) if si else []))
                        pending = []
                elif tn in ("InstUnconditionalBranch", "InstCall",
                            "InstCompareBranch"):
                    prev_key = None
                out.append(inst)
            assert not pending
            bb.instructions[:] = out
    return removed


def _build(with_bias):
    import concourse.bass as bass
    import concourse.mybir as mybir
    import concourse.tile as tile
    from concourse import bacc

    f32 = mybir.dt.float32
    bf16 = mybir.dt.bfloat16

    nc = bacc.Bacc(None, target_bir_lowering=False)

    # weights arrive pre-arranged in SBUF layout (contiguous 2-4KB rows
    # per partition -> fast DMA descriptors)
    xt = nc.dram_tensor("xt", [C, T], bf16, kind="ExternalInput")
    wq = nc.dram_tensor("wq", [128, KT * GC], bf16, kind="ExternalInput")
    wk = nc.dram_tensor("wk", [128, KT * GC], bf16, kind="ExternalInput")
    wv = nc.dram_tensor("wv", [128, KT * GC], bf16, kind="ExternalInput")
    wp = nc.dram_tensor("wp", [128, 2 * C], bf16, kind="ExternalInput")
    if with_bias:
        bq = nc.dram_tensor("bq", [128, 2], f32, kind="ExternalInput")
        bk = nc.dram_tensor("bk", [128, 2], f32, kind="ExternalInput")
        bv = nc.dram_tensor("bv", [1, GC], f32, kind="ExternalInput")
    y = nc.dram_tensor("y", [T, C], f32, kind="ExternalOutput")

    with tile.TileContext(nc) as tc:
        with (
            tc.tile_pool(name="ins", bufs=1) as ins,
            tc.tile_pool(name="big", bufs=1) as bigp,
            tc.tile_pool(name="work", bufs=4) as work,
            tc.tile_pool(name="numsb", bufs=2) as numsb,
            tc.tile_pool(name="psa", bufs=1, space="PSUM") as psa,
            tc.tile_pool(name="psb", bufs=1, space="PSUM") as psb,
            tc.tile_pool(name="psav", bufs=1, space="PSUM") as psav,
            tc.tile_pool(name="psy", bufs=1, space="PSUM") as psy,
            tc.tile_pool(name="dram", bufs=8, space="DRAM") as dpool,
        ):
            # ---- input staging; spread over both HWDGE rings + SWDGE so
            # transfers overlap and the first q/k matmuls start ~8us in ----
            wq_sb = ins.tile([128, KT, GC], bf16, tag="wq")
            wk_sb = ins.tile([128, KT, GC], bf16, tag="wk")
            wv_sb = ins.tile([128, KT, GC], bf16, tag="wv")
            wp_sb = ins.tile([128, 2, C], bf16, tag="wp")
            xt_sb = ins.tile([128, KT, T], bf16, tag="xt")
            # chunks arrive in DESCENDING kt order, matching the descending
            # kt-stagger of the accumulation groups below: PE runs gap-free
            # from the first chunk instead of waiting for the last
            for kt in (7, 5, 3, 1):
                nc.sync.dma_start(xt_sb[:, kt, :], xt[kt * 128:(kt + 1) * 128, :])
            nc.scalar.dma_start(wq_sb[:], wq.rearrange("p (a n) -> p a n", a=KT))
            nc.scalar.dma_start(wk_sb[:], wk.rearrange("p (a n) -> p a n", a=KT))
            for kt in (6, 4, 2, 0):
                nc.scalar.dma_start(xt_sb[:, kt, :], xt[kt * 128:(kt + 1) * 128, :])
            nc.gpsimd.dma_start(wv_sb[:], wv.rearrange("p (a n) -> p a n", a=KT))
            nc.gpsimd.dma_start(wp_sb[:], wp.rearrange("p (a n) -> p a n", a=2))
            if with_bias:
                bq_sb = ins.tile([128, 2], f32, tag="bq")
                bk_sb = ins.tile([128, 2], f32, tag="bk")
                bv_sb = ins.tile([128, GC], f32, tag="bv")
                nc.gpsimd.dma_start(bq_sb[:], bq[:])
                nc.gpsimd.dma_start(bk_sb[:], bk[:])
                nc.gpsimd.dma_start(bv_sb[:], bv[0:1, :].to_broadcast([128, GC]))

            # ---- q/k/v projections. Accumulation groups rotate over 4 psum
            # slots (sc x2, av, py); 2 MMs per kt share one weight load ----
            qt_sb = bigp.tile([128, 2, T], bf16, tag="qt")
            kt_sb = bigp.tile([128, 2, T], bf16, tag="kt")
            _slot = [0]

            def qkv_psum():
                i = _slot[0] % 4
                _slot[0] += 1
                pool = (psa, psb, psav, psy)[i]
                tag = ("sca", "scb", "av", "py")[i]
                return pool.tile([128, 1024], f32, tag=tag, name="pqkv")

            def qk_group(mt):
                for src_sb, dst_sb, which in (
                    (wq_sb, qt_sb, "q"), (wk_sb, kt_sb, "k")):
                    for tsp in range(2):
                        pq = qkv_psum()
                        # descending kt from a late-arriving chunk: PE holds
                        # off until DMA nearly done, then runs gap-free (HAM
                        # stays warm once started)
                        kts = [(KT - 1 - _slot[0] - i) % KT for i in range(KT)]
                        for i, kt in enumerate(kts):
                            for half in range(2):
                                nc.tensor.matmul(
                                    pq[:, half * 512:(half + 1) * 512],
                                    src_sb[:, kt, mt * 128:(mt + 1) * 128],
                                    xt_sb[:, kt,
                                          tsp * 1024 + half * 512:
                                          tsp * 1024 + (half + 1) * 512],
                                    start=(i == 0), stop=(i == KT - 1),
                                )
                        dst = dst_sb[:, mt, tsp * 1024:(tsp + 1) * 1024]
                        if with_bias:
                            bias_sb = bq_sb if which == "q" else bk_sb
                            nc.vector.tensor_scalar_add(
                                dst, pq[:], bias_sb[:, mt:mt + 1])
                        else:
                            nc.vector.tensor_copy(dst, pq[:])

            qk_group(0)

            # ---- v projection (+ ones column for the softmax denominator) ----
            vhat_sb = bigp.tile([128, TT, HG, HD + 1], bf16, tag="vhat")
            nc.vector.memset(vhat_sb[:, :, :, HD:HD + 1], 1.0)
            for tt in range(TT):
                pv = qkv_psum()
                kts = [(tt + i) % KT for i in range(KT)]
                for i, kt in enumerate(kts):
                    nc.tensor.matmul(
                        pv[:, 0:GC],
                        xt_sb[:, kt, tt * 128:(tt + 1) * 128],
                        wv_sb[:, kt, :],
                        start=(i == 0), stop=(i == KT - 1),
                    )
                if with_bias:
                    nc.vector.tensor_add(
                        vhat_sb[:, tt, :, 0:HD], pv[:, 0:GC], bv_sb[:])
                else:
                    nc.vector.tensor_copy(vhat_sb[:, tt, :, 0:HD], pv[:, 0:GC])

            qk_group(1)

            # ---- attention; proj groups interleave into the qc1 stream so
            # PE fills the ACT-bound bubble and y writeback overlaps ----
            # attn output packed as head pairs on full 128 partitions so the
            # proj matmuls contract K=128: even head -> partitions 0-63
            # (mul writes directly), odd head -> 64-127 via SBUF-SBUF DMA
            attn2_sb = bigp.tile([128, 2, T], bf16, tag="attn2")

            def attn_head(h):
                mt, off = h // 2, (h % 2) * 64
                pava = psav.tile([128, 1024], f32, tag="av", name="pava")
                pavb = psy.tile([128, 1024], f32, tag="py", name="pavb")
                for tk in range(TT):
                    psca = psa.tile([128, 1024], f32, tag="sca", name="psca")
                    pscb = psb.tile([128, 1024], f32, tag="scb", name="pscb")
                    # 4 scores MMs share one kT weight load (ldw dedup)
                    for qc, psc in ((0, psca), (1, pscb)):
                        for half in range(2):
                            nc.tensor.matmul(
                                psc[:, half * 512:(half + 1) * 512],
                                kt_sb[off:off + 64, mt, tk * 128:(tk + 1) * 128],
                                qt_sb[off:off + 64, mt,
                                      qc * QW + half * 512:
                                      qc * QW + (half + 1) * 512],
                                start=True, stop=True,
                            )
                    eta = work.tile([128, QW], bf16, tag="expt")
                    nc.scalar.activation(
                        eta[:], psca[:], mybir.ActivationFunctionType.Exp,
                        bias=0.0, scale=0.125)
                    etb = work.tile([128, QW], bf16, tag="expt")
                    nc.scalar.activation(
                        etb[:], pscb[:], mybir.ActivationFunctionType.Exp,
                        bias=0.0, scale=0.125)
                    # 4 AV MMs share one vhat weight load
                    for et, pav in ((eta, pava), (etb, pavb)):
                        for half in range(2):
                            nc.tensor.matmul(
                                pav[0:65, half * 512:(half + 1) * 512],
                                vhat_sb[:, tk, h, :],
                                et[:, half * 512:(half + 1) * 512],
                                start=(tk == 0), stop=(tk == TT - 1),
                            )
                for qc, pav in ((0, pava), (1, pavb)):
                    num = numsb.tile([65, QW], f32, tag="num", name="num")
                    nc.vector.tensor_copy(num[:], pav[0:65, :])
                    nc.vector.reciprocal(num[64:65, :], num[64:65, :])
                    dscr = dpool.tile([1, QW], f32, tag="den", name="dscr")
                    nc.gpsimd.dma_start(dscr[:], num[64:65, :])
                    rbc = numsb.tile([64, QW], f32, tag="rbc", name="rbc")
                    nc.gpsimd.dma_start(
                        rbc[:], dscr[0:1, :].to_broadcast([64, QW]))
                    if h % 2 == 0:
                        nc.vector.tensor_mul(
                            attn2_sb[0:64, h // 2, qc * QW:(qc + 1) * QW],
                            num[0:64, :], rbc[:])
                    else:
                        odd = numsb.tile([64, QW], bf16, tag="odd", name="odd")
                        nc.vector.tensor_mul(odd[:], num[0:64, :], rbc[:])
                        nc.sync.dma_start(
                            attn2_sb[64:128, h // 2, qc * QW:(qc + 1) * QW],
                            odd[:])

            def proj_tt(tt):
                py = qkv_psum()
                for p in range(2):
                    for ns in range(2):
                        nc.tensor.matmul(
                            py[:, ns * 512:(ns + 1) * 512],
                            attn2_sb[:, p, tt * 128:(tt + 1) * 128],
                            wp_sb[:, p, ns * 512:(ns + 1) * 512],
                            start=(p == 0), stop=(p == 1),
                        )
                ysb = work.tile([128, 1024], f32, tag="ysb")
                nc.vector.tensor_copy(ysb[:], py[:])
                nc.sync.dma_start(y[tt * 128:(tt + 1) * 128, :], ysb[:])

            for h in range(HG):
                attn_head(h)
            for tt in range(TT):
                proj_tt(tt)

    nc.compile()
    _dedup_ldweights(nc)
    return nc


def _get_nc(with_bias):
    key = ("nc", with_bias)
    if key not in _cache:
        _cache[key] = _build(with_bias)
    return _cache[key]


def _sbuf_weight_layout(w, p):
    """[a*p, n] -> [p, a*n] matching sbuf tile [p, a, n]."""
    a = w.shape[0] // p
    return np.ascontiguousarray(
        w.reshape(a, p, w.shape[1]).transpose(1, 0, 2).reshape(p, -1))


def make_in_maps(x, w_qkv, b_qkv, w_proj, with_bias):
    bf = ml_dtypes.bfloat16
    x = np.asarray(x, dtype=np.float32)
    w_qkv = np.asarray(w_qkv, dtype=np.float32)
    b_qkv = np.asarray(b_qkv, dtype=np.float32)
    w_proj = np.asarray(w_proj, dtype=np.float32)
    in_maps = []
    for c in range(NCORES):
        b, g = divmod(c, HG)
        cols = slice(g * GC, (g + 1) * GC)
        m = {
            "xt": np.ascontiguousarray(x[b].T).astype(bf),
            "wq": _sbuf_weight_layout(
                w_qkv[:, 0 * C:1 * C][:, cols].astype(bf), 128),
            "wk": _sbuf_weight_layout(
                w_qkv[:, 1 * C:2 * C][:, cols].astype(bf), 128),
            "wv": _sbuf_weight_layout(
                w_qkv[:, 2 * C:3 * C][:, cols].astype(bf), 128),
            "wp": _sbuf_weight_layout(
                w_proj[g * GC:(g + 1) * GC, :].astype(bf), 128),
        }
        if with_bias:
            m["bq"] = np.ascontiguousarray(
                b_qkv[0 * C:1 * C][cols].reshape(2, 128).T).astype(np.float32)
            m["bk"] = np.ascontiguousarray(
                b_qkv[1 * C:2 * C][cols].reshape(2, 128).T).astype(np.float32)
            m["bv"] = np.ascontiguousarray(
                b_qkv[2 * C:3 * C][cols].reshape(1, GC)).astype(np.float32)
        in_maps.append(m)
    return in_maps


def gather(results, b_proj):
    b_proj = np.asarray(b_proj, dtype=np.float32)
    out = np.zeros((B, T, C), dtype=np.float32)
    for c in range(NCORES):
        b = c // HG
        out[b] += results[c]["y"]
    out += b_proj[None, None, :]
    return out


def kernel(x, w_qkv, b_qkv, w_proj, b_proj, _trace=False, _tmpdir=None):
    from concourse import bass_utils
    with_bias = bool(np.any(np.asarray(b_qkv)))
    nc = _get_nc(with_bias)
    in_maps = make_in_maps(x, w_qkv, b_qkv, w_proj, with_bias)
    res = bass_utils.run_bass_kernel_spmd(
        nc, in_maps, core_ids=list(range(NCORES)), trace=_trace,
        tmpdir=_tmpdir)
    _cache["last_result"] = res
    return gather(res.results, b_proj)
